# revision 11
# baseline (speedup 1.0000x reference)
"""Trainium2 Bass kernel for the ClassifierGenerator dense transformer.

Strategy: pure data-parallel over batch. B=8, 8 NeuronCores -> one batch
element per core; all weights replicated, no collectives. Per core the whole
network (4 embed convs, 4 attention blocks of 4 heads each, 4 head convs,
masked log-softmax) runs out of SBUF.

Numerics: bf16 matmul operands with fp32 PSUM accumulation, bf16 stored
activations, fp32 attention-normalization and log-softmax tail. Validated
host-side against the fp32 reference: absmax error ~6e-4 on logits.

Attention per head (field length F=1024, query length Q=1024, NKEY=32,
NVAL=128), all in [channel, position] layout:
  fk_all [4*32, F], qk_all [4*32, Q]   (heads concatenated on partitions;
                                        1/sqrt(32) folded into fk weights)
  fvT_all [F, 4*128]                   (computed directly transposed;
                                        bias via an extra ones-row matmul)
  per head, over 8 field chunks of 128:
    z[128f, Q]   = fk_h[:, chunk].T @ qk_h          (PSUM, fp32)
    w            = min(exp(z - 30), 1)              (ACT exp, DVE min; equals
                                                     exp(clip(z, -30, 30)) *
                                                     e^-30 -- the global e^-30
                                                     cancels in normalization;
                                                     the lower clip is vacuous
                                                     because each column's max
                                                     is ~30 after clipping)
    denom[1, Q] += ones.T @ w                       (PSUM accumulate)
    out[128v, Q]+= fvT_chunk.T @ w                  (PSUM accumulate)
  r = 1/denom, broadcast across partitions (GPSIMD), out *= r.
"""

import numpy as np
import ml_dtypes

import concourse.bacc as bacc
import concourse.mybir as mybir
import concourse.tile as tile
from concourse import bass_utils

BF16 = mybir.dt.bfloat16
F32 = mybir.dt.float32
AF = mybir.ActivationFunctionType
ALU = mybir.AluOpType

L = 1024
NET = 512
CLS = 16
NKEY = 32
NCORES = 8
NH = 2          # free-dim halves of 512 (PSUM bank = 512 fp32)
HALF = 512

BLOCKS = ("attn1", "attn2", "attn3", "attn4")

# conv name -> Cout; all have 4 K-chunks of 128 (emb1a/emb1b are special-cased)
CONV_SPECS = {
    "w2a": NET, "w2b": NET,
    "w3a": NET, "w3b": NET,
    "w4a": NET, "w4b": NET,
    "w5a": NET, "w5b": NET,
    "w6": NET, "w7": NET, "w8": NET,
    "w9": CLS,
}


def _input_specs():
    """name -> (shape, mybir dtype) for every DRAM input of the per-core kernel."""
    sp = {
        "memA": ([128, L], BF16),
        "memB": ([16, L], BF16),
        "testx": ([128, L], BF16),
        "tailb": ([CLS, 1], F32),
        "w1b_a": ([128, 1, NET], BF16),
        "w1b_b": ([16, 1, NET], BF16),
        "b1b": ([128, 4], F32),
        "w1a": ([128, 1, NET], BF16),
        "b1a": ([128, 4], F32),
    }
    for name, cout in CONV_SPECS.items():
        sp[name] = ([128, 4, cout], BF16)
        if name != "w9":  # w9 bias folded into tailb
            sp["b" + name[1:]] = ([128, cout // 128], F32)
    for i in range(4):
        sp[f"fkw{i}"] = ([128, 4, 128], BF16)
        sp[f"fkb{i}"] = ([64, 2], F32)
        sp[f"qkw{i}"] = ([128, 4, 128], BF16)
        sp[f"qkb{i}"] = ([64, 2], F32)
        sp[f"fvw{i}"] = ([128, 4, NET], BF16)
        sp[f"fvb{i}"] = ([1, NET], BF16)
    return sp


def build():
    nc = bacc.Bacc("TRN2", target_bir_lowering=False, debug=False)
    specs = _input_specs()
    dram = {n: nc.dram_tensor(n, shape, dt, kind="ExternalInput")
            for n, (shape, dt) in specs.items()}
    out_d = nc.dram_tensor("out", [CLS, L], F32, kind="ExternalOutput")

    from contextlib import ExitStack
    with tile.TileContext(nc) as tc, ExitStack() as es:
        pools = dict(
            wpool=es.enter_context(tc.tile_pool(name="weights", bufs=1)),
            state=es.enter_context(tc.tile_pool(name="state", bufs=1)),
            act2=es.enter_context(tc.tile_pool(name="act2", bufs=2)),
            act1=es.enter_context(tc.tile_pool(name="act1", bufs=1)),
            wring=es.enter_context(tc.tile_pool(name="wring", bufs=3)),
            pz=es.enter_context(tc.tile_pool(name="pz", bufs=4, space="PSUM")),
            pacc=es.enter_context(tc.tile_pool(name="pacc", bufs=2, space="PSUM")),
            pd=es.enter_context(tc.tile_pool(name="pd", bufs=2, space="PSUM")),
        )
        _body(nc, dram, out_d, pools)

    nc.compile()
    return nc


def _body(nc, dram, out_d, pools):
    wpool, state = pools["wpool"], pools["state"]
    act2, act1, wring = pools["act2"], pools["act1"], pools["wring"]
    pz, pacc, pd = pools["pz"], pools["pacc"], pools["pd"]

    # --- load weights + inputs into SBUF ---
    sb = {}
    for n, t in dram.items():
        sbt = wpool.tile(list(t.shape), t.dtype, tag=n)
        nc.sync.dma_start(out=sbt[:], in_=t[:])
        sb[n] = sbt

    ones128 = state.tile([128, 1], BF16, tag="ones128")   # lhsT for denom sums
    nc.vector.memset(ones128[:], 1.0)
    ones1 = state.tile([1, 128], BF16, tag="ones1")       # lhsT for fv bias row
    nc.vector.memset(ones1[:], 1.0)
    ones16 = state.tile([16, 128], F32, tag="ones16")     # lhsT for tail sum
    nc.vector.memset(ones16[:], 1.0)
    neg30 = state.tile([128, 1], F32, tag="neg30")        # exp bias
    nc.vector.memset(neg30[:], -30.0)

    def conv(src, n_kc, wname, epilogue, cout=NET):
        """1x1 conv: for each out-chunk mo and half nh, accumulate n_kc matmuls
        into PSUM then run epilogue(ps, mp, mo, nh). src(kc, mo, nh, mp) returns
        (lhsT, rhs)."""
        for mo in range(max(cout // 128, 1)):
            mp = min(cout - mo * 128, 128)
            for nh in range(NH):
                ps = pz.tile([128, HALF], F32, tag="z")
                for kc in range(n_kc):
                    lhsT, rhs = src(kc, mo, nh, mp)
                    nc.tensor.matmul(ps[:mp, :], lhsT, rhs,
                                     start=(kc == 0), stop=(kc == n_kc - 1))
                epilogue(ps, mp, mo, nh)

    def std_src(x_tile, wname):
        w = sb[wname]
        def f(kc, mo, nh, mp):
            return (w[:, kc, mo * 128:mo * 128 + mp],
                    x_tile[:, kc, nh * HALF:(nh + 1) * HALF])
        return f

    def act_epi(dst, bias, func, scale=1.0):
        def f(ps, mp, mo, nh):
            nc.scalar.activation(dst[:, mo, nh * HALF:(nh + 1) * HALF],
                                 ps[:mp, :], func,
                                 bias=bias[:, mo:mo + 1], scale=scale)
        return f

    def resid_epi(dst, bias, res):
        def f(ps, mp, mo, nh):
            sl = slice(nh * HALF, (nh + 1) * HALF)
            nc.vector.scalar_tensor_tensor(
                dst[:, mo, sl], ps[:mp, :], bias[:, mo:mo + 1], res[:, mo, sl],
                op0=ALU.add, op1=ALU.add)
        return f

    # --- embeddings ---
    h = act2.tile([128, 4, L], BF16, tag="h")

    def mem_src(kc, mo, nh, mp):
        if kc == 0:
            return (sb["w1b_a"][:, 0, mo * 128:(mo + 1) * 128],
                    sb["memA"][:, nh * HALF:(nh + 1) * HALF])
        return (sb["w1b_b"][:, 0, mo * 128:(mo + 1) * 128],
                sb["memB"][:, nh * HALF:(nh + 1) * HALF])

    conv(mem_src, 2, "w1b_a", act_epi(h, sb["b1b"], AF.Relu))
    x = act2.tile([128, 4, L], BF16, tag="x")
    conv(std_src(h, "w2b"), 4, "w2b", act_epi(x, sb["b2b"], AF.Relu, scale=10.0))

    h2 = act2.tile([128, 4, L], BF16, tag="h")

    def test_src(kc, mo, nh, mp):
        return (sb["w1a"][:, 0, mo * 128:(mo + 1) * 128],
                sb["testx"][:, nh * HALF:(nh + 1) * HALF])

    conv(test_src, 1, "w1a", act_epi(h2, sb["b1a"], AF.Relu))
    y = act1.tile([128, 4, L], BF16, tag="y")
    conv(std_src(h2, "w2a"), 4, "w2a", act_epi(y, sb["b2a"], AF.Relu, scale=10.0))

    def mha(i, F_t, Q_t):
        """4-head attention block i: field F_t, query Q_t -> zm [128,4,L] bf16."""
        # heads packed in pairs: tile A = heads 0,1 / tile B = heads 2,3 at
        # partitions 0-63 (matmul operands may only base at partition 0/32/64)
        fkp = [state.tile([64, L], BF16, tag="fkA", name="fkA"),
               state.tile([64, L], BF16, tag="fkB", name="fkB")]
        qkp = [state.tile([64, L], BF16, tag="qkA", name="qkA"),
               state.tile([64, L], BF16, tag="qkB", name="qkB")]
        fvT = state.tile([128, 8, HALF], BF16, tag="fvT")
        for wname, bname, src, dsts in ((f"fkw{i}", f"fkb{i}", F_t, fkp),
                                        (f"qkw{i}", f"qkb{i}", Q_t, qkp)):
            w, b = sb[wname], sb[bname]
            for h2 in range(2):
                for nh in range(NH):
                    ps = pz.tile([128, HALF], F32, tag="z")
                    for kc in range(4):
                        nc.tensor.matmul(ps[:64, :],
                                         w[:, kc, h2 * 64:h2 * 64 + 64],
                                         src[:, kc, nh * HALF:(nh + 1) * HALF],
                                         start=(kc == 0), stop=(kc == 3))
                    nc.scalar.activation(dsts[h2][:, nh * HALF:(nh + 1) * HALF],
                                         ps[:64, :], AF.Identity,
                                         bias=b[:, h2:h2 + 1], scale=1.0)
        fvw, fvb = sb[f"fvw{i}"], sb[f"fvb{i}"]
        for fc in range(8):
            ps = pz.tile([128, HALF], F32, tag="z")
            for kc in range(4):
                nc.tensor.matmul(ps[:], F_t[:, kc, fc * 128:(fc + 1) * 128],
                                 fvw[:, kc, :], start=(kc == 0), stop=False)
            nc.tensor.matmul(ps[:], ones1[:], fvb[:], start=False, stop=True)
            nc.vector.tensor_copy(fvT[:, fc, :], ps[:])

        zm = act1.tile([128, 4, L], BF16, tag="zm")
        for hd in range(4):
            fkt, qkt = fkp[hd // 2], qkp[hd // 2]
            p0 = (hd % 2) * NKEY
            ops = [pacc.tile([128, HALF], F32, tag="o", name=f"o{i}_{hd}_{nh}")
                   for nh in range(NH)]
            dps = [pd.tile([1, HALF], F32, tag="d", name=f"d{i}_{hd}_{nh}")
                   for nh in range(NH)]

            def consume(w_t, c):
                for nh in range(NH):
                    sl = slice(nh * HALF, (nh + 1) * HALF)
                    nc.tensor.matmul(dps[nh][:], ones128[:, 0:1], w_t[:, sl],
                                     start=(c == 0), stop=(c == 7))
                    nc.tensor.matmul(ops[nh][:],
                                     fvT[:, c, hd * 128:(hd + 1) * 128],
                                     w_t[:, sl], start=(c == 0), stop=(c == 7))

            w_prev = None
            for fc in range(8):
                w_t = wring.tile([128, L], BF16, tag="w")
                for nh in range(NH):
                    zp = pz.tile([128, HALF], F32, tag="z")
                    nc.tensor.matmul(
                        zp[:], fkt[p0:p0 + NKEY, fc * 128:(fc + 1) * 128],
                        qkt[p0:p0 + NKEY, nh * HALF:(nh + 1) * HALF])
                    nc.scalar.activation(w_t[:, nh * HALF:(nh + 1) * HALF],
                                         zp[:], AF.Exp, bias=neg30[:, 0:1],
                                         scale=1.0)
                nc.vector.tensor_scalar_min(w_t[:], w_t[:], 1.0)
                # software pipeline: consume w(fc-1) so PE overlaps ACT/DVE
                if w_prev is not None:
                    consume(w_prev, fc - 1)
                w_prev = w_t
            consume(w_prev, 7)

            r = state.tile([1, L], F32, tag="r")
            rb = state.tile([128, L], F32, tag="rb")
            for nh in range(NH):
                nc.vector.reciprocal(r[:, nh * HALF:(nh + 1) * HALF], dps[nh][:])
            nc.gpsimd.partition_broadcast(rb[:], r[:])
            for nh in range(NH):
                sl = slice(nh * HALF, (nh + 1) * HALF)
                nc.vector.tensor_tensor(zm[:, hd, sl], ops[nh][:], rb[:, sl],
                                        op=ALU.mult)
        return zm

    # --- transformer blocks ---
    zm = mha(0, x, x)
    u = act2.tile([128, 4, L], BF16, tag="h")
    conv(std_src(zm, "w3a"), 4, "w3a", act_epi(u, sb["b3a"], AF.Relu))
    x2 = act2.tile([128, 4, L], BF16, tag="x")
    conv(std_src(u, "w3b"), 4, "w3b", resid_epi(x2, sb["b3b"], x))

    zm = mha(1, x2, x2)
    u = act2.tile([128, 4, L], BF16, tag="h")
    conv(std_src(zm, "w4a"), 4, "w4a", act_epi(u, sb["b4a"], AF.Relu))
    xm = act1.tile([128, 4, L], BF16, tag="xm")
    conv(std_src(u, "w4b"), 4, "w4b", resid_epi(xm, sb["b4b"], x2))

    zm = mha(2, xm, y)
    u = act2.tile([128, 4, L], BF16, tag="h")
    conv(std_src(zm, "w5a"), 4, "w5a", act_epi(u, sb["b5a"], AF.Relu))
    z5 = act1.tile([128, 4, L], BF16, tag="z5")
    conv(std_src(u, "w5b"), 4, "w5b", resid_epi(z5, sb["b5b"], y))

    zm = mha(3, xm, z5)
    u = act2.tile([128, 4, L], BF16, tag="h")
    conv(std_src(zm, "w6"), 4, "w6", act_epi(u, sb["b6"], AF.Relu))
    u2 = act2.tile([128, 4, L], BF16, tag="x")
    conv(std_src(u, "w7"), 4, "w7", act_epi(u2, sb["b7"], AF.Relu))
    u3 = act2.tile([128, 4, L], BF16, tag="h")
    conv(std_src(u2, "w8"), 4, "w8", act_epi(u3, sb["b8"], AF.Relu))

    # --- tail: logits = conv(u3, w9) + b9 + mask; out = log_softmax(logits) ---
    ys = state.tile([CLS, L], F32, tag="ys")
    es = state.tile([CLS, L], F32, tag="es")
    ls = state.tile([CLS, L], F32, tag="ls")
    outs = state.tile([CLS, L], F32, tag="outs")

    def tail_epi(ps, mp, mo, nh):
        sl = slice(nh * HALF, (nh + 1) * HALF)
        nc.vector.tensor_scalar(ys[:, sl], ps[:mp, :], sb["tailb"][:, 0:1], None,
                                op0=ALU.add)
        nc.scalar.activation(es[:, sl], ys[:, sl], AF.Exp)

    conv(std_src(u3, "w9"), 4, "w9", tail_epi, cout=CLS)
    for nh in range(NH):
        sl = slice(nh * HALF, (nh + 1) * HALF)
        ps = pz.tile([128, HALF], F32, tag="z")
        nc.tensor.matmul(ps[:], ones16[:], es[:, sl])   # fp32: exact col sums
        nc.scalar.activation(ls[:, sl], ps[:CLS, :], AF.Ln)
        nc.vector.tensor_sub(outs[:, sl], ys[:, sl], ls[:, sl])
    nc.sync.dma_start(out=out_d[:], in_=outs[:])


# ---------------------------------------------------------------------------
# host side
# ---------------------------------------------------------------------------

def _bf(a):
    return np.ascontiguousarray(np.asarray(a, np.float32).astype(ml_dtypes.bfloat16))


def _f32(a):
    return np.ascontiguousarray(np.asarray(a, np.float32))


def _pack_wt(W):
    """W [Cout, Cin] -> lhsT layout [128, Cin//128, Cout] (bf16)."""
    WT = np.asarray(W, np.float32).T  # [Cin, Cout]
    cin, cout = WT.shape
    return _bf(WT.reshape(cin // 128, 128, cout).transpose(1, 0, 2))


def _pack_bias(b):
    b = np.asarray(b, np.float32)
    n = b.shape[0] // 128
    return _f32(b.reshape(n, 128).T)


def _prep_shared(params):
    shared = {}
    W1b = np.asarray(params["emb1b"]["W"], np.float32).T  # [144, 512]
    shared["w1b_a"] = _bf(W1b[:128][None].transpose(1, 0, 2))
    shared["w1b_b"] = _bf(W1b[128:144][None].transpose(1, 0, 2))
    shared["b1b"] = _pack_bias(params["emb1b"]["b"])
    shared["w1a"] = _bf(np.asarray(params["emb1a"]["W"], np.float32).T[None]
                        .transpose(1, 0, 2))
    shared["b1a"] = _pack_bias(params["emb1a"]["b"])

    name_map = {"w2a": "emb2a", "w2b": "emb2b",
                "w3a": "emb3a", "w3b": "emb3b", "w4a": "emb4a", "w4b": "emb4b",
                "w5a": "emb5a", "w5b": "emb5b", "w6": "emb6", "w7": "emb7",
                "w8": "emb8", "w9": "emb9"}
    for wn, pn in name_map.items():
        shared[wn] = _pack_wt(params[pn]["W"])
        if wn == "w9":
            continue
        b = np.asarray(params[pn]["b"], np.float32)
        if wn in ("w2a", "w2b"):
            b = b * 10.0  # ACT computes relu(10*psum + bias): pre-scale bias
        shared["b" + wn[1:]] = _pack_bias(b)

    s = np.float32(1.0 / np.sqrt(NKEY))
    for i, blk in enumerate(BLOCKS):
        cat = lambda key, sub: np.concatenate(
            [np.asarray(params[blk + h][key][sub], np.float32) for h in "abcd"],
            axis=0)
        shared[f"fkw{i}"] = _pack_wt(cat("fk", "W") * s)
        shared[f"fkb{i}"] = _f32((cat("fk", "b") * s).reshape(2, 64).T)
        shared[f"qkw{i}"] = _pack_wt(cat("qk", "W"))
        shared[f"qkb{i}"] = _f32(cat("qk", "b").reshape(2, 64).T)
        shared[f"fvw{i}"] = _pack_wt(cat("fv", "W"))
        shared[f"fvb{i}"] = _bf(cat("fv", "b").reshape(1, NET))
    shared["_b9"] = np.asarray(params["emb9"]["b"], np.float32)
    return shared


def make_in_maps(mem, test, classes, params):
    shared = _prep_shared(params)
    b9 = shared.pop("_b9")
    mem = np.asarray(mem, np.float32)
    test = np.asarray(test, np.float32)
    classes = np.asarray(classes)
    in_maps = []
    for b in range(NCORES):
        m = dict(shared)
        m["memA"] = _bf(mem[b, 0, :128, :])
        m["memB"] = _bf(mem[b, 0, 128:144, :])
        m["testx"] = _bf(test[b, 0, :, :])
        mask = np.where(np.arange(CLS) >= int(classes[b]), -30.0, 0.0)
        m["tailb"] = _f32((b9 + mask).reshape(CLS, 1))
        in_maps.append(m)
    return in_maps


_compiled = None


def _get_compiled():
    global _compiled
    if _compiled is None:
        _compiled = build()
    return _compiled


def kernel(mem, test, classes, params, **run_kwargs):
    nc = _get_compiled()
    in_maps = make_in_maps(mem, test, classes, params)
    res = bass_utils.run_bass_kernel_spmd(nc, in_maps,
                                          core_ids=list(range(NCORES)),
                                          **run_kwargs)
    kernel.last_results = res
    out = np.stack([res.results[b]["out"] for b in range(NCORES)], axis=0)
    return out.astype(np.float32)


# revision 15
# speedup vs baseline: 1.4601x; 1.4601x over previous
"""Trainium2 Bass kernel for the ClassifierGenerator dense transformer.

Strategy: pure data-parallel over batch. B=8, 8 NeuronCores -> one batch
element per core; all weights replicated, no collectives. Per core the whole
network (4 embed convs, 4 attention blocks of 4 heads each, 4 head convs,
masked log-softmax) runs out of SBUF.

Numerics: bf16 matmul operands with fp32 PSUM accumulation, bf16 stored
activations, fp32 attention-normalization and log-softmax tail. Validated
host-side against the fp32 reference: absmax error ~6e-4 on logits.

Attention per head (field length F=1024, query length Q=1024, NKEY=32,
NVAL=128), all in [channel, position] layout:
  fk_all [4*32, F], qk_all [4*32, Q]   (heads concatenated on partitions;
                                        1/sqrt(32) folded into fk weights)
  fvT_all [F, 4*128]                   (computed directly transposed;
                                        bias via an extra ones-row matmul)
  per head, over 8 field chunks of 128:
    z[128f, Q]   = fk_h[:, chunk].T @ qk_h          (PSUM, fp32)
    w            = min(exp(z - 30), 1)              (ACT exp, DVE min; equals
                                                     exp(clip(z, -30, 30)) *
                                                     e^-30 -- the global e^-30
                                                     cancels in normalization;
                                                     the lower clip is vacuous
                                                     because each column's max
                                                     is ~30 after clipping)
    denom[1, Q] += ones.T @ w                       (PSUM accumulate)
    out[128v, Q]+= fvT_chunk.T @ w                  (PSUM accumulate)
  r = 1/denom, broadcast across partitions (GPSIMD), out *= r.
"""

import numpy as np
import ml_dtypes

import concourse.bacc as bacc
import concourse.mybir as mybir
import concourse.tile as tile
from concourse import bass_utils

BF16 = mybir.dt.bfloat16
F32 = mybir.dt.float32
AF = mybir.ActivationFunctionType
ALU = mybir.AluOpType

L = 1024
NET = 512
CLS = 16
NKEY = 32
NCORES = 8
NH = 2          # free-dim halves of 512 (PSUM bank = 512 fp32)
HALF = 512

BLOCKS = ("attn1", "attn2", "attn3", "attn4")

# conv name -> Cout; all have 4 K-chunks of 128 (emb1a/emb1b are special-cased)
CONV_SPECS = {
    "w2a": NET, "w2b": NET,
    "w3a": NET, "w3b": NET,
    "w4a": NET, "w4b": NET,
    "w5a": NET, "w5b": NET,
    "w6": NET, "w7": NET, "w8": NET,
    "w9": CLS,
}


def _input_specs():
    """name -> (shape, mybir dtype) for every DRAM input of the per-core kernel."""
    sp = {
        "memA": ([128, L], BF16),
        "memB": ([16, L], BF16),
        "testx": ([128, L], BF16),
        "tailb": ([CLS, 1], F32),
        "w1b_a": ([128, 1, NET], BF16),
        "w1b_b": ([16, 1, NET], BF16),
        "b1b": ([128, 4], F32),
        "w1a": ([128, 1, NET], BF16),
        "b1a": ([128, 4], F32),
    }
    for name, cout in CONV_SPECS.items():
        sp[name] = ([128, 4, cout], BF16)
        if name != "w9":  # w9 bias folded into tailb
            sp["b" + name[1:]] = ([128, cout // 128], F32)
    for i in range(4):
        sp[f"fkw{i}"] = ([128, 4, 128], BF16)
        sp[f"fkb{i}"] = ([64, 2], F32)
        sp[f"qkw{i}"] = ([128, 4, 128], BF16)
        sp[f"qkb{i}"] = ([64, 2], F32)
        sp[f"fvw{i}"] = ([128, 4, NET], BF16)
        sp[f"fvb{i}"] = ([1, NET], BF16)
    return sp


def build():
    nc = bacc.Bacc("TRN2", target_bir_lowering=False, debug=False)
    specs = _input_specs()
    dram = {n: nc.dram_tensor(n, shape, dt, kind="ExternalInput")
            for n, (shape, dt) in specs.items()}
    out_d = nc.dram_tensor("out", [CLS, L], F32, kind="ExternalOutput")

    from contextlib import ExitStack
    with tile.TileContext(nc) as tc, ExitStack() as es:
        pools = dict(
            wpool=es.enter_context(tc.tile_pool(name="weights", bufs=1)),
            state=es.enter_context(tc.tile_pool(name="state", bufs=1)),
            act2=es.enter_context(tc.tile_pool(name="act2", bufs=2)),
            act1=es.enter_context(tc.tile_pool(name="act1", bufs=1)),
            wring=es.enter_context(tc.tile_pool(name="wring", bufs=3)),
            pz=es.enter_context(tc.tile_pool(name="pz", bufs=4, space="PSUM")),
            pacc=es.enter_context(tc.tile_pool(name="pacc", bufs=2, space="PSUM")),
            pd=es.enter_context(tc.tile_pool(name="pd", bufs=2, space="PSUM")),
        )
        _body(nc, dram, out_d, pools)

    nc.compile()
    return nc


def _body(nc, dram, out_d, pools):
    wpool, state = pools["wpool"], pools["state"]
    act2, act1, wring = pools["act2"], pools["act1"], pools["wring"]
    pz, pacc, pd = pools["pz"], pools["pacc"], pools["pd"]

    # --- load weights + inputs into SBUF (embedding-critical tensors first) ---
    sb = {}
    first = ["memA", "w1b_a", "w1b_b", "b1b", "memB", "w2b", "b2b",
             "testx", "w1a", "b1a", "w2a", "b2a"]
    order = first + [n for n in dram if n not in first]
    for n in order:
        t = dram[n]
        sbt = wpool.tile(list(t.shape), t.dtype, tag=n, name=f"sb_{n}")
        nc.sync.dma_start(out=sbt[:], in_=t[:])
        sb[n] = sbt

    # all-ones [128,128] lhsT: denominator matmul with M=128 -> the column sum
    # lands broadcast across all 128 PSUM partitions (and keeps the PE array
    # fully active for the HAM activity monitor)
    onesq = state.tile([128, 128], BF16, tag="onesq")
    nc.vector.memset(onesq[:], 1.0)
    ones1 = state.tile([1, 128], BF16, tag="ones1")       # lhsT for fv bias row
    nc.vector.memset(ones1[:], 1.0)
    ones16 = state.tile([16, 128], F32, tag="ones16")     # lhsT for tail sum
    nc.vector.memset(ones16[:], 1.0)
    neg30 = state.tile([128, 1], F32, tag="neg30")        # exp bias
    nc.vector.memset(neg30[:], -30.0)

    def conv(src, n_kc, wname, epilogue, cout=NET):
        """1x1 conv: for each out-chunk mo and half nh, accumulate n_kc matmuls
        into PSUM then run epilogue(ps, mp, mo, nh). src(kc, mo, nh, mp) returns
        (lhsT, rhs)."""
        for mo in range(max(cout // 128, 1)):
            mp = min(cout - mo * 128, 128)
            for nh in range(NH):
                ps = pz.tile([128, HALF], F32, tag="z")
                for kc in range(n_kc):
                    lhsT, rhs = src(kc, mo, nh, mp)
                    nc.tensor.matmul(ps[:mp, :], lhsT, rhs,
                                     start=(kc == 0), stop=(kc == n_kc - 1))
                epilogue(ps, mp, mo, nh)

    def std_src(x_tile, wname):
        w = sb[wname]
        def f(kc, mo, nh, mp):
            return (w[:, kc, mo * 128:mo * 128 + mp],
                    x_tile[:, kc, nh * HALF:(nh + 1) * HALF])
        return f

    def act_epi(dst, bias, func, scale=1.0):
        def f(ps, mp, mo, nh):
            nc.scalar.activation(dst[:, mo, nh * HALF:(nh + 1) * HALF],
                                 ps[:mp, :], func,
                                 bias=bias[:, mo:mo + 1], scale=scale)
        return f

    def resid_epi(dst, bias, res):
        def f(ps, mp, mo, nh):
            sl = slice(nh * HALF, (nh + 1) * HALF)
            nc.vector.scalar_tensor_tensor(
                dst[:, mo, sl], ps[:mp, :], bias[:, mo:mo + 1], res[:, mo, sl],
                op0=ALU.add, op1=ALU.add)
        return f

    # --- embeddings ---
    h = act2.tile([128, 4, L], BF16, tag="h")

    def mem_src(kc, mo, nh, mp):
        if kc == 0:
            return (sb["w1b_a"][:, 0, mo * 128:(mo + 1) * 128],
                    sb["memA"][:, nh * HALF:(nh + 1) * HALF])
        return (sb["w1b_b"][:, 0, mo * 128:(mo + 1) * 128],
                sb["memB"][:, nh * HALF:(nh + 1) * HALF])

    conv(mem_src, 2, "w1b_a", act_epi(h, sb["b1b"], AF.Relu))
    x = act2.tile([128, 4, L], BF16, tag="x")
    conv(std_src(h, "w2b"), 4, "w2b", act_epi(x, sb["b2b"], AF.Relu, scale=10.0))

    h2 = act2.tile([128, 4, L], BF16, tag="h")

    def test_src(kc, mo, nh, mp):
        return (sb["w1a"][:, 0, mo * 128:(mo + 1) * 128],
                sb["testx"][:, nh * HALF:(nh + 1) * HALF])

    conv(test_src, 1, "w1a", act_epi(h2, sb["b1a"], AF.Relu))
    y = act1.tile([128, 4, L], BF16, tag="y")
    conv(std_src(h2, "w2a"), 4, "w2a", act_epi(y, sb["b2a"], AF.Relu, scale=10.0))

    def mha(i, F_t, Q_t):
        """4-head attention block i: field F_t, query Q_t -> zm [128,4,L] bf16."""
        # heads packed in pairs: tile A = heads 0,1 / tile B = heads 2,3 at
        # partitions 0-63 (matmul operands may only base at partition 0/32/64)
        fkp = [state.tile([64, L], BF16, tag="fkA", name="fkA"),
               state.tile([64, L], BF16, tag="fkB", name="fkB")]
        qkp = [state.tile([64, L], BF16, tag="qkA", name="qkA"),
               state.tile([64, L], BF16, tag="qkB", name="qkB")]
        fvT = state.tile([128, 8, HALF], BF16, tag="fvT")
        for wname, bname, src, dsts in ((f"fkw{i}", f"fkb{i}", F_t, fkp),
                                        (f"qkw{i}", f"qkb{i}", Q_t, qkp)):
            w, b = sb[wname], sb[bname]
            for h2 in range(2):
                for nh in range(NH):
                    ps = pz.tile([128, HALF], F32, tag="z")
                    for kc in range(4):
                        nc.tensor.matmul(ps[:64, :],
                                         w[:, kc, h2 * 64:h2 * 64 + 64],
                                         src[:, kc, nh * HALF:(nh + 1) * HALF],
                                         start=(kc == 0), stop=(kc == 3))
                    nc.scalar.activation(dsts[h2][:, nh * HALF:(nh + 1) * HALF],
                                         ps[:64, :], AF.Identity,
                                         bias=b[:, h2:h2 + 1], scale=1.0)
        fvw, fvb = sb[f"fvw{i}"], sb[f"fvb{i}"]
        for fc in range(8):
            ps = pz.tile([128, HALF], F32, tag="z")
            for kc in range(4):
                nc.tensor.matmul(ps[:], F_t[:, kc, fc * 128:(fc + 1) * 128],
                                 fvw[:, kc, :], start=(kc == 0), stop=False)
            nc.tensor.matmul(ps[:], ones1[:], fvb[:], start=False, stop=True)
            nc.vector.tensor_copy(fvT[:, fc, :], ps[:])

        zm = act1.tile([128, 4, L], BF16, tag="zm")
        # heads processed in row-packed pairs (rows 0-31 / 32-63 issue
        # adjacently and overlap in the PE array); query dim in two 512-column
        # passes so o/d accumulators for both heads of a pair fit in PSUM
        for pr in range(2):
            fkt, qkt = fkp[pr], qkp[pr]
            for qh in range(NH):
                qsl = slice(qh * HALF, (qh + 1) * HALF)
                ops = [pacc.tile([128, HALF], F32, tag="o",
                                 name=f"o{i}_{pr}_{qh}_{sh}") for sh in range(2)]
                dps = [pd.tile([128, HALF], F32, tag="d",
                               name=f"d{i}_{pr}_{qh}_{sh}") for sh in range(2)]

                def consume(w_pair, c):
                    for sh in range(2):
                        hd = pr * 2 + sh
                        nc.tensor.matmul(dps[sh][:], onesq[:], w_pair[sh][:],
                                         start=(c == 0), stop=(c == 7))
                        nc.tensor.matmul(ops[sh][:],
                                         fvT[:, c, hd * 128:(hd + 1) * 128],
                                         w_pair[sh][:], start=(c == 0),
                                         stop=(c == 7))

                w_prev = None
                for fc in range(8):
                    fsl = slice(fc * 128, (fc + 1) * 128)
                    w_pair = [wring.tile([128, HALF], BF16, tag="w",
                                         name=f"w{i}_{pr}_{qh}_{fc}_{sh}")
                              for sh in range(2)]
                    zps = [pz.tile([128, HALF], F32, tag="z",
                                   name=f"z{i}_{pr}_{qh}_{fc}_{sh}")
                           for sh in range(2)]
                    for sh in range(2):  # back-to-back: row groups 0 and 32
                        p0 = sh * NKEY
                        nc.tensor.matmul(zps[sh][:], fkt[p0:p0 + NKEY, fsl],
                                         qkt[p0:p0 + NKEY, qsl])
                    for sh in range(2):
                        nc.scalar.activation(w_pair[sh][:], zps[sh][:], AF.Exp,
                                             bias=neg30[:, 0:1], scale=1.0)
                        nc.vector.tensor_scalar_min(w_pair[sh][:],
                                                    w_pair[sh][:], 1.0)
                    # software pipeline: consume w(fc-1) so PE overlaps ACT/DVE
                    if w_prev is not None:
                        consume(w_prev, fc - 1)
                    w_prev = w_pair
                consume(w_prev, 7)

                for sh in range(2):
                    hd = pr * 2 + sh
                    rb = wring.tile([128, HALF], F32, tag="rb",
                                    name=f"rb{i}_{pr}_{qh}_{sh}")
                    nc.vector.reciprocal_approx_fast(rb[:], dps[sh][:])
                    nc.vector.tensor_tensor(zm[:, hd, qsl], ops[sh][:], rb[:],
                                            op=ALU.mult)
        return zm

    # --- transformer blocks ---
    zm = mha(0, x, x)
    u = act2.tile([128, 4, L], BF16, tag="h")
    conv(std_src(zm, "w3a"), 4, "w3a", act_epi(u, sb["b3a"], AF.Relu))
    x2 = act2.tile([128, 4, L], BF16, tag="x")
    conv(std_src(u, "w3b"), 4, "w3b", resid_epi(x2, sb["b3b"], x))

    zm = mha(1, x2, x2)
    u = act2.tile([128, 4, L], BF16, tag="h")
    conv(std_src(zm, "w4a"), 4, "w4a", act_epi(u, sb["b4a"], AF.Relu))
    xm = act1.tile([128, 4, L], BF16, tag="xm")
    conv(std_src(u, "w4b"), 4, "w4b", resid_epi(xm, sb["b4b"], x2))

    zm = mha(2, xm, y)
    u = act2.tile([128, 4, L], BF16, tag="h")
    conv(std_src(zm, "w5a"), 4, "w5a", act_epi(u, sb["b5a"], AF.Relu))
    z5 = act1.tile([128, 4, L], BF16, tag="z5")
    conv(std_src(u, "w5b"), 4, "w5b", resid_epi(z5, sb["b5b"], y))

    zm = mha(3, xm, z5)
    u = act2.tile([128, 4, L], BF16, tag="h")
    conv(std_src(zm, "w6"), 4, "w6", act_epi(u, sb["b6"], AF.Relu))
    u2 = act2.tile([128, 4, L], BF16, tag="x")
    conv(std_src(u, "w7"), 4, "w7", act_epi(u2, sb["b7"], AF.Relu))
    u3 = act2.tile([128, 4, L], BF16, tag="h")
    conv(std_src(u2, "w8"), 4, "w8", act_epi(u3, sb["b8"], AF.Relu))

    # --- tail: logits = conv(u3, w9) + b9 + mask; out = log_softmax(logits) ---
    ys = state.tile([CLS, L], F32, tag="ys")
    es = state.tile([CLS, L], F32, tag="es")
    ls = state.tile([CLS, L], F32, tag="ls")
    outs = state.tile([CLS, L], F32, tag="outs")

    def tail_epi(ps, mp, mo, nh):
        sl = slice(nh * HALF, (nh + 1) * HALF)
        nc.vector.tensor_scalar(ys[:, sl], ps[:mp, :], sb["tailb"][:, 0:1], None,
                                op0=ALU.add)
        nc.scalar.activation(es[:, sl], ys[:, sl], AF.Exp)

    conv(std_src(u3, "w9"), 4, "w9", tail_epi, cout=CLS)
    for nh in range(NH):
        sl = slice(nh * HALF, (nh + 1) * HALF)
        ps = pz.tile([128, HALF], F32, tag="z")
        nc.tensor.matmul(ps[:], ones16[:], es[:, sl])   # fp32: exact col sums
        nc.scalar.activation(ls[:, sl], ps[:CLS, :], AF.Ln)
        nc.vector.tensor_sub(outs[:, sl], ys[:, sl], ls[:, sl])
    nc.sync.dma_start(out=out_d[:], in_=outs[:])


# ---------------------------------------------------------------------------
# host side
# ---------------------------------------------------------------------------

def _bf(a):
    return np.ascontiguousarray(np.asarray(a, np.float32).astype(ml_dtypes.bfloat16))


def _f32(a):
    return np.ascontiguousarray(np.asarray(a, np.float32))


def _pack_wt(W):
    """W [Cout, Cin] -> lhsT layout [128, Cin//128, Cout] (bf16)."""
    WT = np.asarray(W, np.float32).T  # [Cin, Cout]
    cin, cout = WT.shape
    return _bf(WT.reshape(cin // 128, 128, cout).transpose(1, 0, 2))


def _pack_bias(b):
    b = np.asarray(b, np.float32)
    n = b.shape[0] // 128
    return _f32(b.reshape(n, 128).T)


def _prep_shared(params):
    shared = {}
    W1b = np.asarray(params["emb1b"]["W"], np.float32).T  # [144, 512]
    shared["w1b_a"] = _bf(W1b[:128][None].transpose(1, 0, 2))
    shared["w1b_b"] = _bf(W1b[128:144][None].transpose(1, 0, 2))
    shared["b1b"] = _pack_bias(params["emb1b"]["b"])
    shared["w1a"] = _bf(np.asarray(params["emb1a"]["W"], np.float32).T[None]
                        .transpose(1, 0, 2))
    shared["b1a"] = _pack_bias(params["emb1a"]["b"])

    name_map = {"w2a": "emb2a", "w2b": "emb2b",
                "w3a": "emb3a", "w3b": "emb3b", "w4a": "emb4a", "w4b": "emb4b",
                "w5a": "emb5a", "w5b": "emb5b", "w6": "emb6", "w7": "emb7",
                "w8": "emb8", "w9": "emb9"}
    for wn, pn in name_map.items():
        shared[wn] = _pack_wt(params[pn]["W"])
        if wn == "w9":
            continue
        b = np.asarray(params[pn]["b"], np.float32)
        if wn in ("w2a", "w2b"):
            b = b * 10.0  # ACT computes relu(10*psum + bias): pre-scale bias
        shared["b" + wn[1:]] = _pack_bias(b)

    s = np.float32(1.0 / np.sqrt(NKEY))
    for i, blk in enumerate(BLOCKS):
        cat = lambda key, sub: np.concatenate(
            [np.asarray(params[blk + h][key][sub], np.float32) for h in "abcd"],
            axis=0)
        shared[f"fkw{i}"] = _pack_wt(cat("fk", "W") * s)
        shared[f"fkb{i}"] = _f32((cat("fk", "b") * s).reshape(2, 64).T)
        shared[f"qkw{i}"] = _pack_wt(cat("qk", "W"))
        shared[f"qkb{i}"] = _f32(cat("qk", "b").reshape(2, 64).T)
        shared[f"fvw{i}"] = _pack_wt(cat("fv", "W"))
        shared[f"fvb{i}"] = _bf(cat("fv", "b").reshape(1, NET))
    shared["_b9"] = np.asarray(params["emb9"]["b"], np.float32)
    return shared


def make_in_maps(mem, test, classes, params):
    shared = _prep_shared(params)
    b9 = shared.pop("_b9")
    mem = np.asarray(mem, np.float32)
    test = np.asarray(test, np.float32)
    classes = np.asarray(classes)
    in_maps = []
    for b in range(NCORES):
        m = dict(shared)
        m["memA"] = _bf(mem[b, 0, :128, :])
        m["memB"] = _bf(mem[b, 0, 128:144, :])
        m["testx"] = _bf(test[b, 0, :, :])
        mask = np.where(np.arange(CLS) >= int(classes[b]), -30.0, 0.0)
        m["tailb"] = _f32((b9 + mask).reshape(CLS, 1))
        in_maps.append(m)
    return in_maps


_compiled = None


def _get_compiled():
    global _compiled
    if _compiled is None:
        _compiled = build()
    return _compiled


def kernel(mem, test, classes, params, **run_kwargs):
    nc = _get_compiled()
    in_maps = make_in_maps(mem, test, classes, params)
    res = bass_utils.run_bass_kernel_spmd(nc, in_maps,
                                          core_ids=list(range(NCORES)),
                                          **run_kwargs)
    kernel.last_results = res
    out = np.stack([res.results[b]["out"] for b in range(NCORES)], axis=0)
    return out.astype(np.float32)


# revision 20
# speedup vs baseline: 1.5059x; 1.0314x over previous
"""Trainium2 Bass kernel for the ClassifierGenerator dense transformer.

Strategy: pure data-parallel over batch. B=8, 8 NeuronCores -> one batch
element per core; all weights replicated, no collectives. Per core the whole
network (4 embed convs, 4 attention blocks of 4 heads each, 4 head convs,
masked log-softmax) runs out of SBUF.

Numerics: bf16 matmul operands with fp32 PSUM accumulation, bf16 stored
activations, fp32 attention-normalization and log-softmax tail. Validated
host-side against the fp32 reference: absmax error ~6e-4 on logits.

Attention per head (field length F=1024, query length Q=1024, NKEY=32,
NVAL=128), all in [channel, position] layout:
  fk_all [4*32, F], qk_all [4*32, Q]   (heads concatenated on partitions;
                                        1/sqrt(32) folded into fk weights)
  fvT_all [F, 4*128]                   (computed directly transposed;
                                        bias via an extra ones-row matmul)
  per head, over 8 field chunks of 128:
    z[128f, Q]   = fk_h[:, chunk].T @ qk_h          (PSUM, fp32)
    w            = min(exp(z - 30), 1)              (ACT exp, DVE min; equals
                                                     exp(clip(z, -30, 30)) *
                                                     e^-30 -- the global e^-30
                                                     cancels in normalization;
                                                     the lower clip is vacuous
                                                     because each column's max
                                                     is ~30 after clipping)
    denom[1, Q] += ones.T @ w                       (PSUM accumulate)
    out[128v, Q]+= fvT_chunk.T @ w                  (PSUM accumulate)
  r = 1/denom, broadcast across partitions (GPSIMD), out *= r.
"""

import numpy as np
import ml_dtypes

import concourse.bacc as bacc
import concourse.mybir as mybir
import concourse.tile as tile
from concourse import bass_utils

BF16 = mybir.dt.bfloat16
F32 = mybir.dt.float32
AF = mybir.ActivationFunctionType
ALU = mybir.AluOpType

L = 1024
NET = 512
CLS = 16
NKEY = 32
NCORES = 8
NH = 2          # free-dim halves of 512 (PSUM bank = 512 fp32)
HALF = 512

BLOCKS = ("attn1", "attn2", "attn3", "attn4")

# conv name -> Cout; all have 4 K-chunks of 128 (emb1a/emb1b are special-cased)
CONV_SPECS = {
    "w2a": NET, "w2b": NET,
    "w3a": NET, "w3b": NET,
    "w4a": NET, "w4b": NET,
    "w5a": NET, "w5b": NET,
    "w6": NET, "w7": NET, "w8": NET,
    "w9": CLS,
}


def _input_specs():
    """name -> (shape, mybir dtype) for every DRAM input of the per-core kernel."""
    sp = {
        "memA": ([128, L], BF16),
        "memB": ([16, L], BF16),
        "testx": ([128, L], BF16),
        "tailb": ([CLS, 1], F32),
        "w1b_a": ([128, 1, NET], BF16),
        "w1b_b": ([16, 1, NET], BF16),
        "b1b": ([128, 4], F32),
        "w1a": ([128, 1, NET], BF16),
        "b1a": ([128, 4], F32),
    }
    for name, cout in CONV_SPECS.items():
        sp[name] = ([128, 4, cout], BF16)
        if name != "w9":  # w9 bias folded into tailb
            sp["b" + name[1:]] = ([128, cout // 128], F32)
    for i in range(4):
        sp[f"fkw{i}"] = ([128, 4, 128], BF16)
        sp[f"fkb{i}"] = ([64, 2], F32)
        sp[f"qkw{i}"] = ([128, 4, 128], BF16)
        sp[f"qkb{i}"] = ([64, 2], F32)
        sp[f"fvw{i}"] = ([128, 4, NET], BF16)
    return sp


def build():
    nc = bacc.Bacc("TRN2", target_bir_lowering=False, debug=False)
    specs = _input_specs()
    dram = {n: nc.dram_tensor(n, shape, dt, kind="ExternalInput")
            for n, (shape, dt) in specs.items()}
    out_d = nc.dram_tensor("out", [CLS, L], F32, kind="ExternalOutput")

    from contextlib import ExitStack
    with tile.TileContext(nc) as tc, ExitStack() as es:
        pools = dict(
            wpool=es.enter_context(tc.tile_pool(name="weights", bufs=1)),
            state=es.enter_context(tc.tile_pool(name="state", bufs=1)),
            act2=es.enter_context(tc.tile_pool(name="act2", bufs=2)),
            act1=es.enter_context(tc.tile_pool(name="act1", bufs=1)),
            wring=es.enter_context(tc.tile_pool(name="wring", bufs=3)),
            pz=es.enter_context(tc.tile_pool(name="pz", bufs=4, space="PSUM")),
            pacc=es.enter_context(tc.tile_pool(name="pacc", bufs=2, space="PSUM")),
            pd=es.enter_context(tc.tile_pool(name="pd", bufs=2, space="PSUM")),
        )
        _body(nc, dram, out_d, pools)

    nc.compile()
    return nc


def _body(nc, dram, out_d, pools):
    wpool, state = pools["wpool"], pools["state"]
    act2, act1, wring = pools["act2"], pools["act1"], pools["wring"]
    pz, pacc, pd = pools["pz"], pools["pacc"], pools["pd"]

    # --- load weights + inputs into SBUF (embedding-critical tensors first) ---
    sb = {}
    first = ["memA", "w1b_a", "w1b_b", "b1b", "memB", "w2b", "b2b",
             "testx", "w1a", "b1a", "w2a", "b2a"]
    order = first + [n for n in dram if n not in first]
    for n in order:
        t = dram[n]
        sbt = wpool.tile(list(t.shape), t.dtype, tag=n, name=f"sb_{n}")
        nc.sync.dma_start(out=sbt[:], in_=t[:])
        sb[n] = sbt

    # all-ones [128,128] lhsT: denominator matmul with M=128 -> the column sum
    # lands broadcast across all 128 PSUM partitions (and keeps the PE array
    # fully active for the HAM activity monitor)
    onesq = state.tile([128, 128], BF16, tag="onesq")
    nc.vector.memset(onesq[:], 1.0)
    ones16 = state.tile([16, 128], F32, tag="ones16")     # lhsT for tail sum
    nc.vector.memset(ones16[:], 1.0)
    neg30 = state.tile([128, 1], F32, tag="neg30")        # exp bias
    nc.vector.memset(neg30[:], -30.0)

    def conv(src, n_kc, wname, epilogue, cout=NET):
        """1x1 conv: for each out-chunk mo and half nh, accumulate n_kc matmuls
        into PSUM then run epilogue(ps, mp, mo, nh). src(kc, mo, nh, mp) returns
        (lhsT, rhs)."""
        for mo in range(max(cout // 128, 1)):
            mp = min(cout - mo * 128, 128)
            for nh in range(NH):
                ps = pz.tile([128, HALF], F32, tag="z")
                for kc in range(n_kc):
                    lhsT, rhs = src(kc, mo, nh, mp)
                    nc.tensor.matmul(ps[:mp, :], lhsT, rhs,
                                     start=(kc == 0), stop=(kc == n_kc - 1))
                epilogue(ps, mp, mo, nh)

    def std_src(x_tile, wname):
        w = sb[wname]
        def f(kc, mo, nh, mp):
            return (w[:, kc, mo * 128:mo * 128 + mp],
                    x_tile[:, kc, nh * HALF:(nh + 1) * HALF])
        return f

    def act_epi(dst, bias, func, scale=1.0):
        def f(ps, mp, mo, nh):
            nc.scalar.activation(dst[:, mo, nh * HALF:(nh + 1) * HALF],
                                 ps[:mp, :], func,
                                 bias=bias[:, mo:mo + 1], scale=scale)
        return f

    def resid_epi(dst, bias, res):
        def f(ps, mp, mo, nh):
            sl = slice(nh * HALF, (nh + 1) * HALF)
            nc.vector.scalar_tensor_tensor(
                dst[:, mo, sl], ps[:mp, :], bias[:, mo:mo + 1], res[:, mo, sl],
                op0=ALU.add, op1=ALU.add)
        return f

    # --- embeddings ---
    h = act2.tile([128, 4, L], BF16, tag="h")

    def mem_src(kc, mo, nh, mp):
        if kc == 0:
            return (sb["w1b_a"][:, 0, mo * 128:(mo + 1) * 128],
                    sb["memA"][:, nh * HALF:(nh + 1) * HALF])
        return (sb["w1b_b"][:, 0, mo * 128:(mo + 1) * 128],
                sb["memB"][:, nh * HALF:(nh + 1) * HALF])

    conv(mem_src, 2, "w1b_a", act_epi(h, sb["b1b"], AF.Relu))
    x = act2.tile([128, 4, L], BF16, tag="x")
    conv(std_src(h, "w2b"), 4, "w2b", act_epi(x, sb["b2b"], AF.Relu, scale=10.0))

    h2 = act2.tile([128, 4, L], BF16, tag="h")

    def test_src(kc, mo, nh, mp):
        return (sb["w1a"][:, 0, mo * 128:(mo + 1) * 128],
                sb["testx"][:, nh * HALF:(nh + 1) * HALF])

    conv(test_src, 1, "w1a", act_epi(h2, sb["b1a"], AF.Relu))
    y = act1.tile([128, 4, L], BF16, tag="y")
    conv(std_src(h2, "w2a"), 4, "w2a", act_epi(y, sb["b2a"], AF.Relu, scale=10.0))

    def mha(i, F_t, Q_t):
        """4-head attention block i: field F_t, query Q_t -> zm [128,4,L] bf16."""
        # heads packed in pairs: tile A = heads 0,1 / tile B = heads 2,3 at
        # partitions 0-63 (matmul operands may only base at partition 0/32/64)
        fkp = [state.tile([64, L], BF16, tag="fkA", name="fkA"),
               state.tile([64, L], BF16, tag="fkB", name="fkB")]
        qkp = [state.tile([64, L], BF16, tag="qkA", name="qkA"),
               state.tile([64, L], BF16, tag="qkB", name="qkB")]
        fvT = state.tile([128, 8, HALF], BF16, tag="fvT")
        for wname, bname, src, dsts in ((f"fkw{i}", f"fkb{i}", F_t, fkp),
                                        (f"qkw{i}", f"qkb{i}", Q_t, qkp)):
            w, b = sb[wname], sb[bname]
            for h2 in range(2):
                for nh in range(NH):
                    ps = pz.tile([128, HALF], F32, tag="z")
                    for kc in range(4):
                        nc.tensor.matmul(ps[:64, :],
                                         w[:, kc, h2 * 64:h2 * 64 + 64],
                                         src[:, kc, nh * HALF:(nh + 1) * HALF],
                                         start=(kc == 0), stop=(kc == 3))
                    nc.vector.tensor_scalar(
                        dsts[h2][:, nh * HALF:(nh + 1) * HALF], ps[:64, :],
                        b[:, h2:h2 + 1], None, op0=ALU.add)
        # fv bias omitted: sum(w)/denom == 1, so the bias rides through the
        # attention average exactly and is folded into the next conv's bias
        # host-side (b_next += W_next @ concat(b_fv)).
        fvw = sb[f"fvw{i}"]
        for fc in range(8):
            ps = pz.tile([128, HALF], F32, tag="z")
            for kc in range(4):
                nc.tensor.matmul(ps[:], F_t[:, kc, fc * 128:(fc + 1) * 128],
                                 fvw[:, kc, :], start=(kc == 0), stop=(kc == 3))
            nc.vector.tensor_copy(fvT[:, fc, :], ps[:])

        zm = act1.tile([128, 4, L], BF16, tag="zm")
        # heads processed in row-packed pairs (rows 0-31 / 32-63 issue
        # adjacently and overlap in the PE array); query dim in two 512-column
        # passes so o/d accumulators for both heads of a pair fit in PSUM
        for pr in range(2):
            fkt, qkt = fkp[pr], qkp[pr]
            for qh in range(NH):
                qsl = slice(qh * HALF, (qh + 1) * HALF)
                ops = [pacc.tile([128, HALF], F32, tag="o",
                                 name=f"o{i}_{pr}_{qh}_{sh}") for sh in range(2)]
                dps = [pd.tile([128, HALF], F32, tag="d",
                               name=f"d{i}_{pr}_{qh}_{sh}") for sh in range(2)]

                def consume(w_pair, c):
                    for sh in range(2):
                        hd = pr * 2 + sh
                        nc.tensor.matmul(dps[sh][:], onesq[:], w_pair[sh][:],
                                         start=(c == 0), stop=(c == 7))
                        nc.tensor.matmul(ops[sh][:],
                                         fvT[:, c, hd * 128:(hd + 1) * 128],
                                         w_pair[sh][:], start=(c == 0),
                                         stop=(c == 7))

                w_prev = None
                for fc in range(8):
                    fsl = slice(fc * 128, (fc + 1) * 128)
                    w_pair = [wring.tile([128, HALF], BF16, tag="w",
                                         name=f"w{i}_{pr}_{qh}_{fc}_{sh}")
                              for sh in range(2)]
                    zps = [pz.tile([128, HALF], F32, tag="z",
                                   name=f"z{i}_{pr}_{qh}_{fc}_{sh}")
                           for sh in range(2)]
                    for sh in range(2):  # back-to-back: row groups 0 and 32
                        p0 = sh * NKEY
                        nc.tensor.matmul(zps[sh][:], fkt[p0:p0 + NKEY, fsl],
                                         qkt[p0:p0 + NKEY, qsl])
                    for sh in range(2):
                        nc.scalar.activation(w_pair[sh][:], zps[sh][:], AF.Exp,
                                             bias=neg30[:, 0:1], scale=1.0)
                        nc.vector.tensor_scalar_min(w_pair[sh][:],
                                                    w_pair[sh][:], 1.0)
                    # software pipeline: consume w(fc-1) so PE overlaps ACT/DVE
                    if w_prev is not None:
                        consume(w_prev, fc - 1)
                    w_prev = w_pair
                consume(w_prev, 7)

                for sh in range(2):
                    hd = pr * 2 + sh
                    rb = wring.tile([128, HALF], F32, tag="rb",
                                    name=f"rb{i}_{pr}_{qh}_{sh}")
                    nc.vector.reciprocal_approx_fast(rb[:], dps[sh][:])
                    nc.vector.tensor_tensor(zm[:, hd, qsl], ops[sh][:], rb[:],
                                            op=ALU.mult)
        return zm

    # --- transformer blocks ---
    zm = mha(0, x, x)
    u = act2.tile([128, 4, L], BF16, tag="h")
    conv(std_src(zm, "w3a"), 4, "w3a", act_epi(u, sb["b3a"], AF.Relu))
    x2 = act2.tile([128, 4, L], BF16, tag="x")
    conv(std_src(u, "w3b"), 4, "w3b", resid_epi(x2, sb["b3b"], x))

    zm = mha(1, x2, x2)
    u = act2.tile([128, 4, L], BF16, tag="h")
    conv(std_src(zm, "w4a"), 4, "w4a", act_epi(u, sb["b4a"], AF.Relu))
    xm = act1.tile([128, 4, L], BF16, tag="xm")
    conv(std_src(u, "w4b"), 4, "w4b", resid_epi(xm, sb["b4b"], x2))

    zm = mha(2, xm, y)
    u = act2.tile([128, 4, L], BF16, tag="h")
    conv(std_src(zm, "w5a"), 4, "w5a", act_epi(u, sb["b5a"], AF.Relu))
    z5 = act1.tile([128, 4, L], BF16, tag="z5")
    conv(std_src(u, "w5b"), 4, "w5b", resid_epi(z5, sb["b5b"], y))

    zm = mha(3, xm, z5)
    u = act2.tile([128, 4, L], BF16, tag="h")
    conv(std_src(zm, "w6"), 4, "w6", act_epi(u, sb["b6"], AF.Relu))
    u2 = act2.tile([128, 4, L], BF16, tag="x")
    conv(std_src(u, "w7"), 4, "w7", act_epi(u2, sb["b7"], AF.Relu))
    u3 = act2.tile([128, 4, L], BF16, tag="h")
    conv(std_src(u2, "w8"), 4, "w8", act_epi(u3, sb["b8"], AF.Relu))

    # --- tail: logits = conv(u3, w9) + b9 + mask; out = log_softmax(logits) ---
    ys = state.tile([CLS, L], F32, tag="ys")
    es = state.tile([CLS, L], F32, tag="es")
    ls = state.tile([CLS, L], F32, tag="ls")
    outs = state.tile([CLS, L], F32, tag="outs")

    def tail_epi(ps, mp, mo, nh):
        sl = slice(nh * HALF, (nh + 1) * HALF)
        nc.vector.tensor_scalar(ys[:, sl], ps[:mp, :], sb["tailb"][:, 0:1], None,
                                op0=ALU.add)
        nc.scalar.activation(es[:, sl], ys[:, sl], AF.Exp)

    conv(std_src(u3, "w9"), 4, "w9", tail_epi, cout=CLS)
    for nh in range(NH):
        sl = slice(nh * HALF, (nh + 1) * HALF)
        ps = pz.tile([128, HALF], F32, tag="z")
        nc.tensor.matmul(ps[:], ones16[:], es[:, sl])   # fp32: exact col sums
        nc.scalar.activation(ls[:, sl], ps[:CLS, :], AF.Ln)
        nc.vector.tensor_sub(outs[:, sl], ys[:, sl], ls[:, sl])
    nc.sync.dma_start(out=out_d[:], in_=outs[:])


# ---------------------------------------------------------------------------
# host side
# ---------------------------------------------------------------------------

def _bf(a):
    return np.ascontiguousarray(np.asarray(a, np.float32).astype(ml_dtypes.bfloat16))


def _f32(a):
    return np.ascontiguousarray(np.asarray(a, np.float32))


def _pack_wt(W):
    """W [Cout, Cin] -> lhsT layout [128, Cin//128, Cout] (bf16)."""
    WT = np.asarray(W, np.float32).T  # [Cin, Cout]
    cin, cout = WT.shape
    return _bf(WT.reshape(cin // 128, 128, cout).transpose(1, 0, 2))


def _pack_bias(b):
    b = np.asarray(b, np.float32)
    n = b.shape[0] // 128
    return _f32(b.reshape(n, 128).T)


def _prep_shared(params):
    shared = {}
    W1b = np.asarray(params["emb1b"]["W"], np.float32).T  # [144, 512]
    shared["w1b_a"] = _bf(W1b[:128][None].transpose(1, 0, 2))
    shared["w1b_b"] = _bf(W1b[128:144][None].transpose(1, 0, 2))
    shared["b1b"] = _pack_bias(params["emb1b"]["b"])
    shared["w1a"] = _bf(np.asarray(params["emb1a"]["W"], np.float32).T[None]
                        .transpose(1, 0, 2))
    shared["b1a"] = _pack_bias(params["emb1a"]["b"])

    s = np.float32(1.0 / np.sqrt(NKEY))
    fvb_cat = {}
    for i, blk in enumerate(BLOCKS):
        cat = lambda key, sub: np.concatenate(
            [np.asarray(params[blk + h][key][sub], np.float32) for h in "abcd"],
            axis=0)
        shared[f"fkw{i}"] = _pack_wt(cat("fk", "W") * s)
        shared[f"fkb{i}"] = _f32((cat("fk", "b") * s).reshape(2, 64).T)
        shared[f"qkw{i}"] = _pack_wt(cat("qk", "W"))
        shared[f"qkb{i}"] = _f32(cat("qk", "b").reshape(2, 64).T)
        shared[f"fvw{i}"] = _pack_wt(cat("fv", "W"))
        fvb_cat[i] = cat("fv", "b")  # folded into the next conv's bias below

    name_map = {"w2a": "emb2a", "w2b": "emb2b",
                "w3a": "emb3a", "w3b": "emb3b", "w4a": "emb4a", "w4b": "emb4b",
                "w5a": "emb5a", "w5b": "emb5b", "w6": "emb6", "w7": "emb7",
                "w8": "emb8", "w9": "emb9"}
    # conv that consumes each attention block's output (absorbs the fv bias)
    absorbs = {"w3a": 0, "w4a": 1, "w5a": 2, "w6": 3}
    for wn, pn in name_map.items():
        W = np.asarray(params[pn]["W"], np.float32)
        shared[wn] = _pack_wt(W)
        if wn == "w9":
            continue
        b = np.asarray(params[pn]["b"], np.float32)
        if wn in absorbs:
            b = b + W @ fvb_cat[absorbs[wn]]
        if wn in ("w2a", "w2b"):
            b = b * 10.0  # ACT computes relu(10*psum + bias): pre-scale bias
        shared["b" + wn[1:]] = _pack_bias(b)
    shared["_b9"] = np.asarray(params["emb9"]["b"], np.float32)
    return shared


def make_in_maps(mem, test, classes, params):
    shared = _prep_shared(params)
    b9 = shared.pop("_b9")
    mem = np.asarray(mem, np.float32)
    test = np.asarray(test, np.float32)
    classes = np.asarray(classes)
    in_maps = []
    for b in range(NCORES):
        m = dict(shared)
        m["memA"] = _bf(mem[b, 0, :128, :])
        m["memB"] = _bf(mem[b, 0, 128:144, :])
        m["testx"] = _bf(test[b, 0, :, :])
        mask = np.where(np.arange(CLS) >= int(classes[b]), -30.0, 0.0)
        m["tailb"] = _f32((b9 + mask).reshape(CLS, 1))
        in_maps.append(m)
    return in_maps


_compiled = None


def _get_compiled():
    global _compiled
    if _compiled is None:
        _compiled = build()
    return _compiled


def kernel(mem, test, classes, params, **run_kwargs):
    nc = _get_compiled()
    in_maps = make_in_maps(mem, test, classes, params)
    res = bass_utils.run_bass_kernel_spmd(nc, in_maps,
                                          core_ids=list(range(NCORES)),
                                          **run_kwargs)
    kernel.last_results = res
    out = np.stack([res.results[b]["out"] for b in range(NCORES)], axis=0)
    return out.astype(np.float32)


# revision 22
# speedup vs baseline: 1.5901x; 1.0559x over previous
"""Trainium2 Bass kernel for the ClassifierGenerator dense transformer.

Strategy: pure data-parallel over batch. B=8, 8 NeuronCores -> one batch
element per core; all weights replicated, no collectives. Per core the whole
network (4 embed convs, 4 attention blocks of 4 heads each, 4 head convs,
masked log-softmax) runs out of SBUF.

Numerics: bf16 matmul operands with fp32 PSUM accumulation, bf16 stored
activations, fp32 attention-normalization and log-softmax tail. Validated
host-side against the fp32 reference: absmax error ~6e-4 on logits.

Attention per head (field length F=1024, query length Q=1024, NKEY=32,
NVAL=128), all in [channel, position] layout:
  fk_all [4*32, F], qk_all [4*32, Q]   (heads concatenated on partitions;
                                        1/sqrt(32) folded into fk weights)
  fvT_all [F, 4*128]                   (computed directly transposed;
                                        bias via an extra ones-row matmul)
  per head, over 8 field chunks of 128:
    z[128f, Q]   = fk_h[:, chunk].T @ qk_h          (PSUM, fp32)
    w            = min(exp(z - 30), 1)              (ACT exp, DVE min; equals
                                                     exp(clip(z, -30, 30)) *
                                                     e^-30 -- the global e^-30
                                                     cancels in normalization;
                                                     the lower clip is vacuous
                                                     because each column's max
                                                     is ~30 after clipping)
    denom[1, Q] += ones.T @ w                       (PSUM accumulate)
    out[128v, Q]+= fvT_chunk.T @ w                  (PSUM accumulate)
  r = 1/denom, broadcast across partitions (GPSIMD), out *= r.
"""

import numpy as np
import ml_dtypes

import concourse.bacc as bacc
import concourse.mybir as mybir
import concourse.tile as tile
from concourse import bass_utils

BF16 = mybir.dt.bfloat16
F32 = mybir.dt.float32
AF = mybir.ActivationFunctionType
ALU = mybir.AluOpType

L = 1024
NET = 512
CLS = 16
NKEY = 32
NCORES = 8
NH = 2          # free-dim halves of 512 (PSUM bank = 512 fp32)
HALF = 512

BLOCKS = ("attn1", "attn2", "attn3", "attn4")

# conv name -> Cout; all have 4 K-chunks of 128 (emb1a/emb1b are special-cased)
CONV_SPECS = {
    "w2a": NET, "w2b": NET,
    "w3a": NET, "w3b": NET,
    "w4a": NET, "w4b": NET,
    "w5a": NET, "w5b": NET,
    "w6": NET, "w7": NET, "w8": NET,
    "w9": CLS,
}


def _input_specs():
    """name -> (shape, mybir dtype) for every DRAM input of the per-core kernel."""
    sp = {
        "memA": ([128, L], BF16),
        "memB": ([16, L], BF16),
        "testx": ([128, L], BF16),
        "tailb": ([CLS, 1], F32),
        "w1b_a": ([128, 1, NET], BF16),
        "w1b_b": ([16, 1, NET], BF16),
        "b1b": ([128, 4], F32),
        "w1a": ([128, 1, NET], BF16),
        "b1a": ([128, 4], F32),
    }
    for name, cout in CONV_SPECS.items():
        sp[name] = ([128, 4, cout], BF16)
        if name != "w9":  # w9 bias folded into tailb
            sp["b" + name[1:]] = ([128, cout // 128], F32)
    for i in range(4):
        sp[f"fkw{i}"] = ([128, 4, 128], BF16)
        sp[f"fkb{i}"] = ([64, 2], F32)
        sp[f"qkw{i}"] = ([128, 4, 128], BF16)
        sp[f"qkb{i}"] = ([64, 2], F32)
        sp[f"fvw{i}"] = ([128, 4, NET], BF16)
    return sp


def build():
    nc = bacc.Bacc("TRN2", target_bir_lowering=False, debug=False)
    specs = _input_specs()
    dram = {n: nc.dram_tensor(n, shape, dt, kind="ExternalInput")
            for n, (shape, dt) in specs.items()}
    out_d = nc.dram_tensor("out", [CLS, L], F32, kind="ExternalOutput")

    from contextlib import ExitStack
    with tile.TileContext(nc) as tc, ExitStack() as es:
        pools = dict(
            wpool=es.enter_context(tc.tile_pool(name="weights", bufs=1)),
            state=es.enter_context(tc.tile_pool(name="state", bufs=1)),
            act2=es.enter_context(tc.tile_pool(name="act2", bufs=2)),
            act1=es.enter_context(tc.tile_pool(name="act1", bufs=1)),
            wring=es.enter_context(tc.tile_pool(name="wring", bufs=6)),
            pz=es.enter_context(tc.tile_pool(name="pz", bufs=4, space="PSUM")),
            pacc=es.enter_context(tc.tile_pool(name="pacc", bufs=2, space="PSUM")),
            pd=es.enter_context(tc.tile_pool(name="pd", bufs=2, space="PSUM")),
        )
        _body(nc, dram, out_d, pools)

    nc.compile()
    return nc


def _body(nc, dram, out_d, pools):
    wpool, state = pools["wpool"], pools["state"]
    act2, act1, wring = pools["act2"], pools["act1"], pools["wring"]
    pz, pacc, pd = pools["pz"], pools["pacc"], pools["pd"]

    # --- load weights + inputs into SBUF (embedding-critical tensors first) ---
    sb = {}
    first = ["memA", "w1b_a", "w1b_b", "b1b", "memB", "w2b", "b2b",
             "testx", "w1a", "b1a", "w2a", "b2a"]
    order = first + [n for n in dram if n not in first]
    for n in order:
        t = dram[n]
        sbt = wpool.tile(list(t.shape), t.dtype, tag=n, name=f"sb_{n}")
        nc.sync.dma_start(out=sbt[:], in_=t[:])
        sb[n] = sbt

    # all-ones [128,128] lhsT: denominator matmul with M=128 -> the column sum
    # lands broadcast across all 128 PSUM partitions (and keeps the PE array
    # fully active for the HAM activity monitor)
    onesq = state.tile([128, 128], BF16, tag="onesq")
    nc.vector.memset(onesq[:], 1.0)
    ones16 = state.tile([16, 128], F32, tag="ones16")     # lhsT for tail sum
    nc.vector.memset(ones16[:], 1.0)
    neg30 = state.tile([128, 1], F32, tag="neg30")        # exp bias
    nc.vector.memset(neg30[:], -30.0)

    def conv(src, n_kc, wname, epilogue, cout=NET):
        """1x1 conv: for each out-chunk mo and half nh, accumulate n_kc matmuls
        into PSUM then run epilogue(ps, mp, mo, nh). src(kc, mo, nh, mp) returns
        (lhsT, rhs)."""
        for mo in range(max(cout // 128, 1)):
            mp = min(cout - mo * 128, 128)
            for nh in range(NH):
                ps = pz.tile([128, HALF], F32, tag="z")
                for kc in range(n_kc):
                    lhsT, rhs = src(kc, mo, nh, mp)
                    nc.tensor.matmul(ps[:mp, :], lhsT, rhs,
                                     start=(kc == 0), stop=(kc == n_kc - 1))
                epilogue(ps, mp, mo, nh)

    def std_src(x_tile, wname):
        w = sb[wname]
        def f(kc, mo, nh, mp):
            return (w[:, kc, mo * 128:mo * 128 + mp],
                    x_tile[:, kc, nh * HALF:(nh + 1) * HALF])
        return f

    def act_epi(dst, bias, func, scale=1.0):
        def f(ps, mp, mo, nh):
            nc.scalar.activation(dst[:, mo, nh * HALF:(nh + 1) * HALF],
                                 ps[:mp, :], func,
                                 bias=bias[:, mo:mo + 1], scale=scale)
        return f

    def resid_epi(dst, bias, res):
        def f(ps, mp, mo, nh):
            sl = slice(nh * HALF, (nh + 1) * HALF)
            nc.vector.scalar_tensor_tensor(
                dst[:, mo, sl], ps[:mp, :], bias[:, mo:mo + 1], res[:, mo, sl],
                op0=ALU.add, op1=ALU.add)
        return f

    # --- embeddings ---
    h = act2.tile([128, 4, L], BF16, tag="h")

    def mem_src(kc, mo, nh, mp):
        if kc == 0:
            return (sb["w1b_a"][:, 0, mo * 128:(mo + 1) * 128],
                    sb["memA"][:, nh * HALF:(nh + 1) * HALF])
        return (sb["w1b_b"][:, 0, mo * 128:(mo + 1) * 128],
                sb["memB"][:, nh * HALF:(nh + 1) * HALF])

    conv(mem_src, 2, "w1b_a", act_epi(h, sb["b1b"], AF.Relu))
    x = act2.tile([128, 4, L], BF16, tag="x")
    conv(std_src(h, "w2b"), 4, "w2b", act_epi(x, sb["b2b"], AF.Relu, scale=10.0))

    h2 = act2.tile([128, 4, L], BF16, tag="h")

    def test_src(kc, mo, nh, mp):
        return (sb["w1a"][:, 0, mo * 128:(mo + 1) * 128],
                sb["testx"][:, nh * HALF:(nh + 1) * HALF])

    conv(test_src, 1, "w1a", act_epi(h2, sb["b1a"], AF.Relu))
    y = act1.tile([128, 4, L], BF16, tag="y")
    conv(std_src(h2, "w2a"), 4, "w2a", act_epi(y, sb["b2a"], AF.Relu, scale=10.0))

    def mha(i, F_t, Q_t):
        """4-head attention block i: field F_t, query Q_t -> zm [128,4,L] bf16."""
        # heads packed in pairs: tile A = heads 0,1 / tile B = heads 2,3 at
        # partitions 0-63 (matmul operands may only base at partition 0/32/64)
        fkp = [state.tile([64, L], BF16, tag="fkA", name="fkA"),
               state.tile([64, L], BF16, tag="fkB", name="fkB")]
        qkp = [state.tile([64, L], BF16, tag="qkA", name="qkA"),
               state.tile([64, L], BF16, tag="qkB", name="qkB")]
        fvT = state.tile([128, 8, HALF], BF16, tag="fvT")
        for wname, bname, src, dsts in ((f"fkw{i}", f"fkb{i}", F_t, fkp),
                                        (f"qkw{i}", f"qkb{i}", Q_t, qkp)):
            w, b = sb[wname], sb[bname]
            for h2 in range(2):
                for nh in range(NH):
                    ps = pz.tile([128, HALF], F32, tag="z")
                    for kc in range(4):
                        nc.tensor.matmul(ps[:64, :],
                                         w[:, kc, h2 * 64:h2 * 64 + 64],
                                         src[:, kc, nh * HALF:(nh + 1) * HALF],
                                         start=(kc == 0), stop=(kc == 3))
                    nc.vector.tensor_scalar(
                        dsts[h2][:, nh * HALF:(nh + 1) * HALF], ps[:64, :],
                        b[:, h2:h2 + 1], None, op0=ALU.add)
        # fv bias omitted: sum(w)/denom == 1, so the bias rides through the
        # attention average exactly and is folded into the next conv's bias
        # host-side (b_next += W_next @ concat(b_fv)).
        fvw = sb[f"fvw{i}"]
        for fc in range(8):
            ps = pz.tile([128, HALF], F32, tag="z")
            for kc in range(4):
                nc.tensor.matmul(ps[:], F_t[:, kc, fc * 128:(fc + 1) * 128],
                                 fvw[:, kc, :], start=(kc == 0), stop=(kc == 3))
            nc.vector.tensor_copy(fvT[:, fc, :], ps[:])

        zm = act1.tile([128, 4, L], BF16, tag="zm")
        # heads processed in row-packed pairs (rows 0-31 / 32-63 issue
        # adjacently and overlap in the PE array); query dim in two 512-column
        # passes so o/d accumulators for both heads of a pair fit in PSUM
        for pr in range(2):
            fkt, qkt = fkp[pr], qkp[pr]
            for qh in range(NH):
                qsl = slice(qh * HALF, (qh + 1) * HALF)
                ops = [pacc.tile([128, HALF], F32, tag="o",
                                 name=f"o{i}_{pr}_{qh}_{sh}") for sh in range(2)]
                dps = [pd.tile([128, HALF], F32, tag="d",
                               name=f"d{i}_{pr}_{qh}_{sh}") for sh in range(2)]

                def consume(w_pair, c):
                    for sh in range(2):
                        hd = pr * 2 + sh
                        nc.tensor.matmul(dps[sh][:], onesq[:], w_pair[sh][:],
                                         start=(c == 0), stop=(c == 7))
                        nc.tensor.matmul(ops[sh][:],
                                         fvT[:, c, hd * 128:(hd + 1) * 128],
                                         w_pair[sh][:], start=(c == 0),
                                         stop=(c == 7))

                pending = []
                for fc in range(8):
                    fsl = slice(fc * 128, (fc + 1) * 128)
                    w_pair = [wring.tile([128, HALF], BF16, tag="w",
                                         name=f"w{i}_{pr}_{qh}_{fc}_{sh}")
                              for sh in range(2)]
                    zps = [pz.tile([128, HALF], F32, tag="z",
                                   name=f"z{i}_{pr}_{qh}_{fc}_{sh}")
                           for sh in range(2)]
                    for sh in range(2):  # back-to-back: row groups 0 and 32
                        p0 = sh * NKEY
                        nc.tensor.matmul(zps[sh][:], fkt[p0:p0 + NKEY, fsl],
                                         qkt[p0:p0 + NKEY, qsl])
                    for sh in range(2):
                        nc.scalar.activation(w_pair[sh][:], zps[sh][:], AF.Exp,
                                             bias=neg30[:, 0:1], scale=1.0)
                        nc.vector.tensor_scalar_min(w_pair[sh][:],
                                                    w_pair[sh][:], 1.0)
                    # software pipeline (depth 2): consume w(fc-2) so the PE
                    # never waits on this chunk's exp/min
                    pending.append((w_pair, fc))
                    if len(pending) > 2:
                        consume(*pending.pop(0))
                for args in pending:
                    consume(*args)

                for sh in range(2):
                    hd = pr * 2 + sh
                    rb = wring.tile([128, HALF], F32, tag="rb",
                                    name=f"rb{i}_{pr}_{qh}_{sh}")
                    nc.vector.reciprocal_approx_fast(rb[:], dps[sh][:])
                    nc.vector.tensor_tensor(zm[:, hd, qsl], ops[sh][:], rb[:],
                                            op=ALU.mult)
        return zm

    # --- transformer blocks ---
    zm = mha(0, x, x)
    u = act2.tile([128, 4, L], BF16, tag="h")
    conv(std_src(zm, "w3a"), 4, "w3a", act_epi(u, sb["b3a"], AF.Relu))
    x2 = act2.tile([128, 4, L], BF16, tag="x")
    conv(std_src(u, "w3b"), 4, "w3b", resid_epi(x2, sb["b3b"], x))

    zm = mha(1, x2, x2)
    u = act2.tile([128, 4, L], BF16, tag="h")
    conv(std_src(zm, "w4a"), 4, "w4a", act_epi(u, sb["b4a"], AF.Relu))
    xm = act1.tile([128, 4, L], BF16, tag="xm")
    conv(std_src(u, "w4b"), 4, "w4b", resid_epi(xm, sb["b4b"], x2))

    zm = mha(2, xm, y)
    u = act2.tile([128, 4, L], BF16, tag="h")
    conv(std_src(zm, "w5a"), 4, "w5a", act_epi(u, sb["b5a"], AF.Relu))
    z5 = act1.tile([128, 4, L], BF16, tag="z5")
    conv(std_src(u, "w5b"), 4, "w5b", resid_epi(z5, sb["b5b"], y))

    zm = mha(3, xm, z5)
    u = act2.tile([128, 4, L], BF16, tag="h")
    conv(std_src(zm, "w6"), 4, "w6", act_epi(u, sb["b6"], AF.Relu))
    u2 = act2.tile([128, 4, L], BF16, tag="x")
    conv(std_src(u, "w7"), 4, "w7", act_epi(u2, sb["b7"], AF.Relu))
    u3 = act2.tile([128, 4, L], BF16, tag="h")
    conv(std_src(u2, "w8"), 4, "w8", act_epi(u3, sb["b8"], AF.Relu))

    # --- tail: logits = conv(u3, w9) + b9 + mask; out = log_softmax(logits) ---
    ys = state.tile([CLS, L], F32, tag="ys")
    es = state.tile([CLS, L], F32, tag="es")
    ls = state.tile([CLS, L], F32, tag="ls")
    outs = state.tile([CLS, L], F32, tag="outs")

    def tail_epi(ps, mp, mo, nh):
        sl = slice(nh * HALF, (nh + 1) * HALF)
        nc.vector.tensor_scalar(ys[:, sl], ps[:mp, :], sb["tailb"][:, 0:1], None,
                                op0=ALU.add)
        nc.scalar.activation(es[:, sl], ys[:, sl], AF.Exp)

    conv(std_src(u3, "w9"), 4, "w9", tail_epi, cout=CLS)
    for nh in range(NH):
        sl = slice(nh * HALF, (nh + 1) * HALF)
        ps = pz.tile([128, HALF], F32, tag="z")
        nc.tensor.matmul(ps[:], ones16[:], es[:, sl])   # fp32: exact col sums
        nc.scalar.activation(ls[:, sl], ps[:CLS, :], AF.Ln)
        nc.vector.tensor_sub(outs[:, sl], ys[:, sl], ls[:, sl])
    nc.sync.dma_start(out=out_d[:], in_=outs[:])


# ---------------------------------------------------------------------------
# host side
# ---------------------------------------------------------------------------

def _bf(a):
    return np.ascontiguousarray(np.asarray(a, np.float32).astype(ml_dtypes.bfloat16))


def _f32(a):
    return np.ascontiguousarray(np.asarray(a, np.float32))


def _pack_wt(W):
    """W [Cout, Cin] -> lhsT layout [128, Cin//128, Cout] (bf16)."""
    WT = np.asarray(W, np.float32).T  # [Cin, Cout]
    cin, cout = WT.shape
    return _bf(WT.reshape(cin // 128, 128, cout).transpose(1, 0, 2))


def _pack_bias(b):
    b = np.asarray(b, np.float32)
    n = b.shape[0] // 128
    return _f32(b.reshape(n, 128).T)


def _prep_shared(params):
    shared = {}
    W1b = np.asarray(params["emb1b"]["W"], np.float32).T  # [144, 512]
    shared["w1b_a"] = _bf(W1b[:128][None].transpose(1, 0, 2))
    shared["w1b_b"] = _bf(W1b[128:144][None].transpose(1, 0, 2))
    shared["b1b"] = _pack_bias(params["emb1b"]["b"])
    shared["w1a"] = _bf(np.asarray(params["emb1a"]["W"], np.float32).T[None]
                        .transpose(1, 0, 2))
    shared["b1a"] = _pack_bias(params["emb1a"]["b"])

    s = np.float32(1.0 / np.sqrt(NKEY))
    fvb_cat = {}
    for i, blk in enumerate(BLOCKS):
        cat = lambda key, sub: np.concatenate(
            [np.asarray(params[blk + h][key][sub], np.float32) for h in "abcd"],
            axis=0)
        shared[f"fkw{i}"] = _pack_wt(cat("fk", "W") * s)
        shared[f"fkb{i}"] = _f32((cat("fk", "b") * s).reshape(2, 64).T)
        shared[f"qkw{i}"] = _pack_wt(cat("qk", "W"))
        shared[f"qkb{i}"] = _f32(cat("qk", "b").reshape(2, 64).T)
        shared[f"fvw{i}"] = _pack_wt(cat("fv", "W"))
        fvb_cat[i] = cat("fv", "b")  # folded into the next conv's bias below

    name_map = {"w2a": "emb2a", "w2b": "emb2b",
                "w3a": "emb3a", "w3b": "emb3b", "w4a": "emb4a", "w4b": "emb4b",
                "w5a": "emb5a", "w5b": "emb5b", "w6": "emb6", "w7": "emb7",
                "w8": "emb8", "w9": "emb9"}
    # conv that consumes each attention block's output (absorbs the fv bias)
    absorbs = {"w3a": 0, "w4a": 1, "w5a": 2, "w6": 3}
    for wn, pn in name_map.items():
        W = np.asarray(params[pn]["W"], np.float32)
        shared[wn] = _pack_wt(W)
        if wn == "w9":
            continue
        b = np.asarray(params[pn]["b"], np.float32)
        if wn in absorbs:
            b = b + W @ fvb_cat[absorbs[wn]]
        if wn in ("w2a", "w2b"):
            b = b * 10.0  # ACT computes relu(10*psum + bias): pre-scale bias
        shared["b" + wn[1:]] = _pack_bias(b)
    shared["_b9"] = np.asarray(params["emb9"]["b"], np.float32)
    return shared


def make_in_maps(mem, test, classes, params):
    shared = _prep_shared(params)
    b9 = shared.pop("_b9")
    mem = np.asarray(mem, np.float32)
    test = np.asarray(test, np.float32)
    classes = np.asarray(classes)
    in_maps = []
    for b in range(NCORES):
        m = dict(shared)
        m["memA"] = _bf(mem[b, 0, :128, :])
        m["memB"] = _bf(mem[b, 0, 128:144, :])
        m["testx"] = _bf(test[b, 0, :, :])
        mask = np.where(np.arange(CLS) >= int(classes[b]), -30.0, 0.0)
        m["tailb"] = _f32((b9 + mask).reshape(CLS, 1))
        in_maps.append(m)
    return in_maps


_compiled = None


def _get_compiled():
    global _compiled
    if _compiled is None:
        _compiled = build()
    return _compiled


def kernel(mem, test, classes, params, **run_kwargs):
    nc = _get_compiled()
    in_maps = make_in_maps(mem, test, classes, params)
    res = bass_utils.run_bass_kernel_spmd(nc, in_maps,
                                          core_ids=list(range(NCORES)),
                                          **run_kwargs)
    kernel.last_results = res
    out = np.stack([res.results[b]["out"] for b in range(NCORES)], axis=0)
    return out.astype(np.float32)
